# revision 1
# baseline (speedup 1.0000x reference)
"""Non-local block (B=8, C=256, H=W=56) as a Bass/Tile kernel on 8 trn2 NeuronCores.

Sharding: pure data parallelism — core i computes sample i end-to-end
(attention is per-sample, weights replicated). kernel() takes the full
inputs, builds per-core input maps, runs the SPMD Bass program, and
stacks the per-core outputs.

Per-core math (xf = x[i] reshaped [C, N], N = H*W = 3136, CH = 128):
  theta = w_theta @ xf          [CH, N]   (bf16)
  phi   = w_phi   @ xf          [CH, N]   (bf16)
  gT    = (w_g @ xf)^T          [N, CH]   (fp8 e4m3, m-major)
  S_T[m, n] = (phi^T theta)[m, n]; E = exp(S_T - 3)        (fp8 e4m3)
  y[c, n] = sum_m gT[m, c] E[m, n]   (fp8 DoubleRow, PSUM-accumulated)
  d[n]    = sum_m E[m, n]            (ones-matmul, fp8 DoubleRow)
  out = w_z @ (y / d) + xf      [C, N]
The exp bias (-3) keeps E inside fp8 range and cancels exactly in y/d.

Structure: attention runs over 512-wide n-chunks x 13 m-units (12
DoubleRow pairs + 64-row tail). Each pair's two score blocks share one
[128, 1024] PSUM tile so a single ACT instruction exps the whole unit,
and its fp8 output [128, 2, 512] is exactly the DoubleRow rhs access
pattern. Scores run two units ahead of PV/d through 3 rotating PSUM
slots (lookahead-2) so the ScalarE exp pipe never starves — this is
worth ~15% end to end. Projections for the first unit are emitted up
front; the rest interleave into the first chunk's unit loop. Each
chunk's normalize/z/residual/store epilogue is deferred and emitted
after the next chunk's first scores so it overlaps. The final 64-wide
n-chunk packs 8 m-blocks of scores per PSUM bank (one exp per 8 blocks).
Stores are row-split 4-way across the sync/gpsimd/scalar queues (whole
1KB DMA packets, three parallel queues) so the final chunk's store
flight is short. Measured: ~121.5 us per core on trn2 (NTFF exec_time),
scale-relative absmax error ~3.8e-4 vs the fp32 reference.
"""

import os
import sys

import numpy as np

for _p in (
    "/opt/trn_rl_repo",
    "/root/.axon_site",
    "/root/.axon_site/_ro/trn_rl_repo",
    "/root/.axon_site/_ro/pypackages",
):
    if _p not in sys.path and os.path.isdir(_p):
        sys.path.append(_p)

import concourse.bass as bass  # noqa: E402
import concourse.bacc as bacc  # noqa: E402
import concourse.tile as tile  # noqa: E402
from concourse import mybir  # noqa: E402
from concourse.masks import make_identity  # noqa: E402

B, C, H, W = 8, 256, 56, 56
N = H * W  # 3136
CH = C // 2  # 128
P = 128

CW = 512  # attention n-chunk width
# six 512-wide chunks + the 64-wide tail last (tiny final epilogue)
CHUNKS = [(i * CW, CW) for i in range(6)] + [(6 * CW, N - 6 * CW)]
MB = 25  # m blocks: 24 x 128 + 1 x 64
MB_TAIL = N - 24 * P  # 64
NPAIR = 12  # DoubleRow m-block pairs (0,1)...(22,23); mb 24 is the tail
DMACH = 784  # x DMA / cast chunk
PCH = 392  # projection free-dim chunk, 8 x 392 = 3136
EXP_BIAS = -3.0

F32 = mybir.dt.float32
BF16 = mybir.dt.bfloat16
F8 = mybir.dt.float8e4

N_CORES = 8


def _mm_cols(width):
    """Split a free-dim width into <=512 column pieces (PSUM bank limit)."""
    cols, off = [], 0
    while off < width:
        w = min(512, width - off)
        cols.append((off, w))
        off += w
    return cols


def _kernel_body(tc):
    nc = tc.nc
    x_d = nc.dram_tensor("x", [C, N], F32, kind="ExternalInput").ap()
    wth_d = nc.dram_tensor("w_theta", [CH, C], F32, kind="ExternalInput").ap()
    wph_d = nc.dram_tensor("w_phi", [CH, C], F32, kind="ExternalInput").ap()
    wg_d = nc.dram_tensor("w_g", [CH, C], F32, kind="ExternalInput").ap()
    wz_d = nc.dram_tensor("w_z", [C, CH], F32, kind="ExternalInput").ap()
    out_d = nc.dram_tensor("out", [C, N], F32, kind="ExternalOutput").ap()

    from contextlib import ExitStack

    with ExitStack() as ctx:
        consts = ctx.enter_context(tc.tile_pool(name="consts", bufs=1))
        etp = ctx.enter_context(tc.tile_pool(name="etp", bufs=6))
        rp = ctx.enter_context(tc.tile_pool(name="rp", bufs=3))
        outp = ctx.enter_context(tc.tile_pool(name="outp", bufs=8))
        psum = ctx.enter_context(tc.tile_pool(name="psum", bufs=3, space="PSUM"))
        psum_y = ctx.enter_context(tc.tile_pool(name="psum_y", bufs=1, space="PSUM"))
        psum_d = ctx.enter_context(tc.tile_pool(name="psum_d", bufs=1, space="PSUM"))

        # ---- persistent SBUF tiles ----
        x_f32 = [consts.tile([P, N], F32, tag=f"x{h}", name=f"x{h}") for h in range(2)]
        x_bf = [
            consts.tile([P, N], BF16, tag=f"xb{h}", name=f"xb{h}") for h in range(2)
        ]
        theta = consts.tile([P, N], BF16, tag="theta", name="theta")
        phi = consts.tile([P, N], BF16, tag="phi", name="phi")
        gT = consts.tile([P, MB * P], F8, tag="gT", name="gT")  # [m_local, mb*128+c]
        ynorm = consts.tile([P, N], BF16, tag="ynorm", name="ynorm")
        e_tail = consts.tile([P, CW], F8, tag="e_tail", name="e_tail")
        identity = consts.tile([P, P], BF16, tag="identity", name="identity")
        ones8 = consts.tile([P, 2, P], F8, tag="ones8", name="ones8")
        expb = consts.tile([P, 1], F32, tag="expb", name="expb")
        w_raw = {
            k: consts.tile([CH, C], F32, tag=f"wraw_{k}", name=f"wraw_{k}")
            for k in ("th", "ph", "g")
        }
        w_bf = {
            k: consts.tile([CH, C], BF16, tag=f"wbf_{k}", name=f"wbf_{k}")
            for k in ("th", "ph", "g")
        }
        wz_raw = [
            consts.tile([P, CH], F32, tag=f"wzraw{h}", name=f"wzraw{h}")
            for h in range(2)
        ]
        wz_bf = [
            consts.tile([P, CH], BF16, tag=f"wzbf{h}", name=f"wzbf{h}")
            for h in range(2)
        ]
        wT = {
            k: [
                consts.tile([P, P], BF16, tag=f"wT_{k}{j}", name=f"wT_{k}{j}")
                for j in range(2)
            ]
            for k in ("th", "ph", "g")
        }
        wzT = [
            consts.tile([P, P], BF16, tag=f"wzT{h}", name=f"wzT{h}") for h in range(2)
        ]

        # alternate psum->sbuf copies between DVE and ACT to balance engines
        _copy_tog = [0]

        def alt_copy(out, in_):
            _copy_tog[0] ^= 1
            if _copy_tog[0]:
                nc.vector.tensor_copy(out=out, in_=in_)
            else:
                nc.scalar.copy(out=out, in_=in_)

        # ---- load inputs. One DMA transfer rides one HWDGE queue (~22-40
        # GB/s each), so latency of the first chunk is size/queue-rate: split
        # early chunks into small sub-transfers spread round-robin across
        # engines, in chunk order so the first chunk's packets queue first.
        out_dma_engs = [nc.sync, nc.gpsimd]  # keep output DMAs off the ACT queue
        # chunk 0 split in 4 sub-transfers on the scalar queue (starts issuing
        # earliest) so its data lands ~5us sooner; rest in order on sync
        for h in range(2):
            for s in range(2):
                sl = slice(s * (DMACH // 2), (s + 1) * (DMACH // 2))
                nc.scalar.dma_start(
                    out=x_f32[h][:, sl], in_=x_d[h * P : (h + 1) * P, sl]
                )
        for k, d in (("th", wth_d), ("ph", wph_d), ("g", wg_d)):
            nc.scalar.dma_start(out=w_raw[k][:], in_=d[:, :])
        for h in range(2):
            nc.gpsimd.dma_start(out=wz_raw[h][:], in_=wz_d[h * P : (h + 1) * P, :])
        for ci in range(1, N // DMACH):
            sl = slice(ci * DMACH, (ci + 1) * DMACH)
            for h in range(2):
                nc.sync.dma_start(
                    out=x_f32[h][:, sl], in_=x_d[h * P : (h + 1) * P, sl]
                )

        make_identity(nc, identity)
        nc.vector.memset(expb, EXP_BIAS)
        nc.vector.memset(ones8, 1.0)
        # only the padding regions of gT / e_tail need zeros; keep these off
        # the DVE queue (they'd delay the startup casts)
        nc.gpsimd.memset(gT[:, (MB - 1) * P :], 0.0)
        nc.gpsimd.memset(e_tail, 0.0)

        # ---- weight casts + PE transposes ----
        for k in ("th", "ph", "g"):
            nc.vector.tensor_copy(out=w_bf[k][:], in_=w_raw[k][:])
        for h in range(2):
            nc.vector.tensor_copy(out=wz_bf[h][:], in_=wz_raw[h][:])

        def pe_transpose(dst, src):
            ps = psum.tile([P, P], BF16, tag="s", name="s")
            nc.tensor.transpose(ps[:], src, identity[:])
            nc.vector.tensor_copy(out=dst, in_=ps[:])

        for k in ("th", "ph", "g"):
            for j in range(2):
                pe_transpose(wT[k][j][:], w_bf[k][:, j * P : (j + 1) * P])
        for h in range(2):
            pe_transpose(wzT[h][:], wz_bf[h][:])

        # ---- deferred x casts + projection emitters (interleaved into
        # attention). Everything stays on DVE: the ACT queue must hold only
        # exps once the attention pipeline starts (in-order queue — a cast
        # waiting on a late DMA would block every exp behind it).
        xc_done = [0]

        # 392-wide casts: chunk 0's DMA lands as 392-wide subs, so the first
        # projection can start after the first sub instead of the full 784
        XCH = PCH  # 392

        def ensure_xcast2(upto):
            upto = min(N // XCH, upto)
            while xc_done[0] < upto:
                j = xc_done[0]
                sl = slice(j * XCH, (j + 1) * XCH)
                for h in range(2):
                    nc.vector.tensor_copy(out=x_bf[h][:, sl], in_=x_f32[h][:, sl])
                xc_done[0] = j + 1

        def emit_proj_chunk(wkey, dst, j):
            ensure_xcast2(j + 1)
            sl = slice(j * PCH, (j + 1) * PCH)
            ps = psum.tile([P, PCH], F32, tag="s", name="s")
            nc.tensor.matmul(
                ps[:], wT[wkey][0][:], x_bf[0][:, sl], start=True, stop=False
            )
            nc.tensor.matmul(
                ps[:], wT[wkey][1][:], x_bf[1][:, sl], start=False, stop=True
            )
            nc.vector.tensor_copy(out=dst[:, sl], in_=ps[:])

        def emit_gt_block(mb):
            mw = P if mb < MB - 1 else MB_TAIL
            ensure_xcast2((mb * P + mw - 1) // XCH + 1)
            msl = slice(mb * P, mb * P + mw)
            ps = psum.tile([P, P], F32, tag="s", name="s")
            nc.tensor.matmul(
                ps[:mw, :], x_bf[0][:, msl], wT["g"][0][:], start=True, stop=False
            )
            nc.tensor.matmul(
                ps[:mw, :], x_bf[1][:, msl], wT["g"][1][:], start=False, stop=True
            )
            nc.vector.tensor_copy(out=gT[:mw, mb * P : (mb + 1) * P], in_=ps[:mw, :])

        done = {"th": 0, "ph": 0, "gT": 0}  # chunks/blocks emitted so far

        def ensure(kind, upto):
            """Emit projection work up to (exclusive) index `upto`."""
            while done[kind] < upto:
                j = done[kind]
                if kind == "th":
                    emit_proj_chunk("th", theta, j)
                elif kind == "ph":
                    emit_proj_chunk("ph", phi, j)
                else:
                    emit_gt_block(j)
                done[kind] = j + 1

        NP_CH = N // PCH  # 8

        def phi_chunks_for_cols(cols):
            return min(NP_CH, (cols + PCH - 1) // PCH)

        def theta_chunks_for_cols(cols):
            return min(NP_CH, (cols + PCH - 1) // PCH)

        # upfront: enough for unit 0 of chunk 0
        ensure("th", theta_chunks_for_cols(CW))  # theta cols 0:1024 -> chunks 0..2
        ensure("ph", 1)  # phi cols 0:256
        ensure("gT", 2)  # m-blocks 0,1

        # ---- attention ----
        DR = mybir.MatmulPerfMode.DoubleRow

        def emit_scores_exp(cs, w, unit, interleave):
            """Scores+exp for all m-blocks of this unit; returns the E tile.

            For pair units both m-blocks' scores land in ONE [P, 2*CW] psum
            tile so a single ACT instruction exps the whole unit, and the
            fp8 output layout [P, 2, CW] is exactly the DoubleRow rhs AP."""
            if unit < NPAIR:
                et = etp.tile([P, 2, CW], F8, tag="et", name="et")
                mbs = (2 * unit, 2 * unit + 1)
            else:
                et = e_tail
                mbs = (MB - 1,)
            if interleave:
                # pre-requisites for this unit's scores and the NEXT unit's PV
                ensure("ph", phi_chunks_for_cols((mbs[-1] + 1) * P))
                if unit < NPAIR:
                    ensure("gT", min(MB, 2 * unit + 2))
                else:
                    ensure("gT", MB)
                # drain remaining theta early (needed from chunk 1 onward)
                ensure("th", min(NP_CH, theta_chunks_for_cols(CW) + unit))
            if unit < NPAIR:
                s_ps = psum.tile([P, 2 * CW], F32, tag="s", name="s")
                for j, mb in enumerate(mbs):
                    nc.tensor.matmul(
                        s_ps[:, j * CW : j * CW + w],
                        phi[:, mb * P : (mb + 1) * P],
                        theta[:, cs : cs + w],
                        start=True,
                        stop=True,
                    )
                if w == CW:
                    nc.scalar.activation(
                        out=et.rearrange("p a b -> p (a b)"),
                        in_=s_ps[:],
                        func=mybir.ActivationFunctionType.Exp,
                        bias=expb[:],
                    )
                else:
                    for j in range(2):
                        nc.scalar.activation(
                            out=et[:, j, :w],
                            in_=s_ps[:, j * CW : j * CW + w],
                            func=mybir.ActivationFunctionType.Exp,
                            bias=expb[:],
                        )
            else:
                mb = MB - 1
                s_ps = psum.tile([P, 2 * CW], F32, tag="s", name="s")
                nc.tensor.matmul(
                    s_ps[:MB_TAIL, :w],
                    phi[:, mb * P : mb * P + MB_TAIL],
                    theta[:, cs : cs + w],
                    start=True,
                    stop=True,
                )
                nc.scalar.activation(
                    out=et[:MB_TAIL, :w],
                    in_=s_ps[:MB_TAIL, :w],
                    func=mybir.ActivationFunctionType.Exp,
                    bias=expb[:MB_TAIL],
                )
            return et

        def emit_pv_d(y_ps, d_ps, et, w, unit, first, last):
            if unit < NPAIR:
                gpair = gT[:, 2 * unit * P : (2 * unit + 2) * P].rearrange(
                    "p (k c) -> p k c", k=2
                )
                for off, wdt in _mm_cols(w):
                    nc.tensor.matmul(
                        y_ps[:, off : off + wdt],
                        gpair,
                        et[:, :, off : off + wdt],
                        start=first,
                        stop=last,
                        perf_mode=DR,
                    )
                    nc.tensor.matmul(
                        d_ps[:, off : off + wdt],
                        ones8[:],
                        et[:, :, off : off + wdt],
                        start=first,
                        stop=last,
                        perf_mode=DR,
                    )
            else:
                for off, wdt in _mm_cols(w):
                    nc.tensor.matmul(
                        y_ps[:, off : off + wdt],
                        gT[:, (MB - 1) * P : MB * P],
                        et[:, off : off + wdt],
                        start=first,
                        stop=last,
                    )
                    nc.tensor.matmul(
                        d_ps[:, off : off + wdt],
                        ones8[:, 0, :],
                        et[:, off : off + wdt],
                        start=first,
                        stop=last,
                    )

        def wide_chunk(cs, w, interleave, flush):
            y_ps = psum_y.tile([P, CW], F32, tag="y", name="y")
            d_ps = psum_d.tile([P, CW], F32, tag="d", name="d")
            order = list(range(NPAIR)) + [NPAIR]
            et_q = [emit_scores_exp(cs, w, order[0], interleave)]
            flush()  # previous chunk's epilogue overlaps this chunk's scores
            et_q.append(emit_scores_exp(cs, w, order[1], interleave))
            for idx, unit in enumerate(order):
                first, last = idx == 0, idx == len(order) - 1
                if idx + 2 < len(order):
                    et_q.append(emit_scores_exp(cs, w, order[idx + 2], interleave))
                emit_pv_d(y_ps, d_ps, et_q.pop(0), w, unit, first, last)
            return y_ps, d_ps

        def tail_chunk(cs, w, flush):
            """64-wide n-chunk: pack 8 m-blocks of scores per PSUM bank so a
            single ACT instruction covers 8 exps."""
            y_ps = psum_y.tile([P, CW], F32, tag="y", name="y")
            d_ps = psum_d.tile([P, CW], F32, tag="d", name="d")

            def group_scores_exp(g):
                if g < 3:
                    s8 = psum.tile([P, 8 * w], F32, tag="s", name="s8")
                    et8 = etp.tile([P, 8, w], F8, tag="et", name="et8")
                    for j in range(8):
                        mb = 8 * g + j
                        nc.tensor.matmul(
                            s8[:, j * w : (j + 1) * w],
                            phi[:, mb * P : (mb + 1) * P],
                            theta[:, cs : cs + w],
                            start=True,
                            stop=True,
                        )
                    nc.scalar.activation(
                        out=et8.rearrange("p a b -> p (a b)"),
                        in_=s8[:, : 8 * w],
                        func=mybir.ActivationFunctionType.Exp,
                        bias=expb[:],
                    )
                    return et8
                mb = MB - 1
                s_ps = psum.tile([P, CW], F32, tag="s", name="s")
                nc.tensor.matmul(
                    s_ps[:MB_TAIL, :w],
                    phi[:, mb * P : mb * P + MB_TAIL],
                    theta[:, cs : cs + w],
                    start=True,
                    stop=True,
                )
                nc.scalar.activation(
                    out=e_tail[:MB_TAIL, :w],
                    in_=s_ps[:MB_TAIL, :w],
                    func=mybir.ActivationFunctionType.Exp,
                    bias=expb[:MB_TAIL],
                )
                return e_tail

            et_cur = group_scores_exp(0)
            flush()  # previous chunk's epilogue overlaps this chunk's scores
            for g in range(4):
                et_next = group_scores_exp(g + 1) if g < 3 else None
                if g < 3:
                    for p_i in range(4):
                        pair = 4 * g + p_i
                        gpair = gT[
                            :, 2 * pair * P : (2 * pair + 2) * P
                        ].rearrange("p (k c) -> p k c", k=2)
                        first = pair == 0
                        nc.tensor.matmul(
                            y_ps[:, :w],
                            gpair,
                            et_cur[:, 2 * p_i : 2 * p_i + 2, :],
                            start=first,
                            stop=False,
                            perf_mode=DR,
                        )
                        nc.tensor.matmul(
                            d_ps[:, :w],
                            ones8[:],
                            et_cur[:, 2 * p_i : 2 * p_i + 2, :],
                            start=first,
                            stop=False,
                            perf_mode=DR,
                        )
                else:
                    nc.tensor.matmul(
                        y_ps[:, :w],
                        gT[:, (MB - 1) * P : MB * P],
                        et_cur[:, :w],
                        start=False,
                        stop=True,
                    )
                    nc.tensor.matmul(
                        d_ps[:, :w],
                        ones8[:, 0, :],
                        et_cur[:, :w],
                        start=False,
                        stop=True,
                    )
                et_cur = et_next
            return y_ps, d_ps

        def make_epilogue(ci, cs, w, y_ps, d_ps):
            def _ep():
                # normalize: ynorm = y / d  (d replicated across partitions)
                r_t = rp.tile([P, CW], F32, tag="r", name="r")
                nc.vector.reciprocal_approx_fast(
                    out=r_t[:, :w], in_=d_ps[:, :w]
                )
                nc.vector.tensor_mul(
                    out=ynorm[:, cs : cs + w], in0=y_ps[:, :w], in1=r_t[:, :w]
                )
                # z = w_z @ ynorm + x -> out. h=0 reuses the d slot (freed
                # after the recip), h=1 the y slot.
                for h, ztag in ((0, "d"), (1, "y")):
                    zpool = psum_y if ztag == "y" else psum_d
                    z_ps = zpool.tile([P, CW], F32, tag=ztag, name="zps")
                    nc.tensor.matmul(
                        z_ps[:, :w],
                        wzT[h][:],
                        ynorm[:, cs : cs + w],
                        start=True,
                        stop=True,
                    )
                    o_t = outp.tile([P, CW], F32, tag="o", name="o")
                    nc.vector.tensor_add(
                        out=o_t[:, :w], in0=z_ps[:, :w], in1=x_f32[h][:, cs : cs + w]
                    )
                    half = (w + 1) // 2
                    for s, (so, sw) in enumerate(((0, half), (half, w - half))):
                        if sw <= 0:
                            continue
                        eng = out_dma_engs[(h + ci + s) % 2]
                        eng.dma_start(
                            out=out_d[h * P : (h + 1) * P, cs + so : cs + so + sw],
                            in_=o_t[:, so : so + sw],
                        )

            return _ep

        pending = [None]

        def flush():
            if pending[0] is not None:
                pending[0]()
                pending[0] = None

        for ci, (cs, w) in enumerate(CHUNKS):
            if w > 64:
                y_ps, d_ps = wide_chunk(cs, w, interleave=(ci == 0), flush=flush)
            else:
                y_ps, d_ps = tail_chunk(cs, w, flush=flush)
            pending[0] = make_epilogue(ci, cs, w, y_ps, d_ps)
        flush()

        assert done == {"th": NP_CH, "ph": NP_CH, "gT": MB}, done


_NC_CACHE = None


def build_nc():
    global _NC_CACHE
    if _NC_CACHE is None:
        nc = bacc.Bacc("TRN2", target_bir_lowering=False, debug=False)
        with tile.TileContext(nc) as tc:
            _kernel_body(tc)
        nc.compile()
        _NC_CACHE = nc
    return _NC_CACHE


def kernel(x, w_theta, w_phi, w_g, w_z, trace=False):
    assert x.shape == (B, C, H, W), x.shape
    nc = build_nc()
    from concourse.bass_utils import run_bass_kernel_spmd

    shared = {
        "w_theta": np.ascontiguousarray(w_theta, dtype=np.float32),
        "w_phi": np.ascontiguousarray(w_phi, dtype=np.float32),
        "w_g": np.ascontiguousarray(w_g, dtype=np.float32),
        "w_z": np.ascontiguousarray(w_z, dtype=np.float32),
    }
    in_maps = [
        dict(shared, x=np.ascontiguousarray(x[i].reshape(C, N), dtype=np.float32))
        for i in range(N_CORES)
    ]
    res = run_bass_kernel_spmd(
        nc, in_maps, core_ids=list(range(N_CORES)), trace=trace
    )
    out = np.stack([res.results[i]["out"].reshape(C, H, W) for i in range(N_CORES)])
    kernel.last_result = res
    return out


kernel.last_result = None

